# revision 1
# baseline (speedup 1.0000x reference)
"""Trainium2 Bass kernel for the DeepFuzzyCMean loss.

loss = GAMMA * sum_{n,k} u[n,k]^2 * ||x[n] - v[k]||^2
     = GAMMA * ( t1 + sum_k c[k]*|v_k|^2 - 2*sum_{k,d} W[k,d]*v[k,d] )
  t1 = sum_n (sum_k u2[n,k]) * |x_n|^2,  c = colsum(u2),  W = u2^T @ x

Device formulation (per 128x(t*rows) tile, contraction over n in PSUM):
  u2  = square(u)                       (ACT)
  xsq = [x*x | 1]                       (DVE square + preset ones col)
  MM1: acc_w[64,128] += u2_b^T @ x_b    (W)
  MM2: acc_q[64,129] += u2_b^T @ xsq_b  ([t1 block | c])
Host combines the per-core [64,257] partials with v in float64.

Wire format: x, u cast to fp16 on host (memory-bound; halves HBM traffic,
rel err ~1e-5). u is pre-scaled by 64 so u^2=4096u^2 avoids fp16 subnormals;
host divides partials by 4096. t=16 consecutive rows map to one SBUF
partition so every DMA is a fully contiguous block (full line rate).

Raw-bass implementation (manual semaphores, standalone sequencer waits —
walrus limits embedded per-instruction sync waits, so waits live on their
own queue slots). B-deep double buffering; data-parallel over N across 8
NeuronCores with a host all-reduce.
"""

import sys
import types
from contextlib import ExitStack

import numpy as np

import concourse.bass as bass
from concourse import mybir
from concourse.bass_utils import run_bass_kernel_spmd

# run_bass_kernel_spmd(trace=True) under axon imports antenv.axon_hooks,
# which this container lacks; stub it so a BASS_TRACE env var can't crash us.
try:
    import antenv.axon_hooks  # noqa: F401
except ImportError:
    try:
        import antenv

        _stub = types.ModuleType("antenv.axon_hooks")
        _stub.get_axon_ntff_profile_hook = lambda: None
        sys.modules["antenv.axon_hooks"] = _stub
        antenv.axon_hooks = _stub
    except ImportError:
        pass

GAMMA = 1e-06
N, K, D = 262144, 64, 128
NCORES = 8
N_CORE = N // NCORES
P = 128
H = D // 2
OUT_W = 2 * D + 1  # [W | t1 block | c] = 257
USCALE = 64.0      # u pre-scale; partials carry USCALE^2 = 4096

LAST_RESULTS = None
_NC_CACHE = {}


def build_nc(n_rows=N_CORE, t=16, nbuf=6, num_devices=NCORES, reps=1):
    """reps>1 repeats the full sweep inside one NEFF (re-reading the same
    DRAM) — used only for differential hardware timing."""
    assert n_rows % (P * t) == 0
    iters = (n_rows // (P * t)) * reps
    n_rows_data = n_rows
    assert iters >= nbuf
    f16 = mybir.dt.float16
    f32 = mybir.dt.float32
    w2 = D + 1

    nc = bass.Bass("TRN2", num_devices=num_devices)
    x_d = nc.dram_tensor("x", [n_rows, D], f16, kind="ExternalInput")
    u_d = nc.dram_tensor("u", [n_rows, K], f16, kind="ExternalInput")
    out_d = nc.dram_tensor("out", [K, OUT_W], f32, kind="ExternalOutput")

    with ExitStack() as ctx:
        xt = [ctx.enter_context(nc.sbuf_tensor(f"xt{j}", [P, t * D], f16)) for j in range(nbuf)]
        ur = [ctx.enter_context(nc.sbuf_tensor(f"ur{j}", [P, t * K], f16)) for j in range(nbuf)]
        u2 = [ctx.enter_context(nc.sbuf_tensor(f"u2{j}", [P, t * K], f16)) for j in range(nbuf)]
        xsq = [ctx.enter_context(nc.sbuf_tensor(f"xsq{j}", [P, t * w2], f16)) for j in range(nbuf)]
        res = ctx.enter_context(nc.sbuf_tensor("res", [K, OUT_W], f32))
        acc_w = ctx.enter_context(nc.psum_tensor([K, D], f32))
        acc_q = ctx.enter_context(nc.psum_tensor([K, w2], f32))

        s_dx = [ctx.enter_context(nc.semaphore(f"s_dx{j}")) for j in range(nbuf)]
        s_du = [ctx.enter_context(nc.semaphore(f"s_du{j}")) for j in range(nbuf)]
        s_act = ctx.enter_context(nc.semaphore("s_act"))
        s_dve = ctx.enter_context(nc.semaphore("s_dve"))
        s_pe = ctx.enter_context(nc.semaphore("s_pe"))
        s_res = ctx.enter_context(nc.semaphore("s_res"))
        s_do = ctx.enter_context(nc.semaphore("s_do"))

        block = ctx.enter_context(nc.Block())

        data_iters = n_rows_data // (P * t)

        @block.sync
        def _(sync):
            for i in range(iters):
                j = i % nbuf
                r = (i % data_iters) * P * t
                e = i // nbuf
                if i >= nbuf:
                    # slot j free: PE done with xt/u2/xq, DVE done with xt,
                    # ACT done with ur of iteration i-nbuf; the slot's own
                    # previous DMAs completed long ago (consumers saw them) —
                    # these waits are immediately satisfied but keep the
                    # per-sem increments ordered for the race checker.
                    sync.wait_ge(s_pe, i - nbuf + 1)
                    sync.wait_ge(s_dve, i - nbuf + 1)
                    sync.wait_ge(s_act, i - nbuf + 1)
                    sync.wait_ge(s_dx[j], 16 * e)
                    sync.wait_ge(s_du[j], 16 * e)
                x_src = x_d[r : r + P * t, :].rearrange("(p b) d -> p (b d)", p=P)
                u_src = u_d[r : r + P * t, :].rearrange("(p b) k -> p (b k)", p=P)
                sync.dma_start(out=xt[j][:, :], in_=x_src).then_inc(s_dx[j], 16)
                sync.dma_start(out=ur[j][:, :], in_=u_src).then_inc(s_du[j], 16)
            sync.wait_ge(s_res, 1)
            sync.dma_start(out=out_d[:, :], in_=res[:, :]).then_inc(s_do, 16)
            sync.wait_ge(s_do, 16)

        @block.scalar
        def _(scalar):
            for i in range(iters):
                j = i % nbuf
                if i >= nbuf:
                    scalar.wait_ge(s_pe, i - nbuf + 1)  # u2 slot reader
                scalar.wait_ge(s_du[j], 16 * (i // nbuf + 1))
                scalar.activation(
                    out=u2[j][:, :],
                    in_=ur[j][:, :],
                    func=mybir.ActivationFunctionType.Square,
                ).then_inc(s_act)
            # tail: psum -> sbuf -> (sync engine DMAs it out)
            scalar.wait_ge(s_pe, iters)
            scalar.copy(res[:, 0:D], acc_w[:, :])
            scalar.copy(res[:, D:OUT_W], acc_q[:, :]).then_inc(s_res)

        @block.vector
        def _(vector):
            # ones columns are static: set once per buffer, never overwritten
            for j in range(nbuf):
                xsq3 = xsq[j][:, :].rearrange("p (b c) -> p b c", b=t)
                vector.memset(xsq3[:, :, D : D + 1], 1.0)
            for i in range(iters):
                j = i % nbuf
                if i >= nbuf:
                    vector.wait_ge(s_pe, i - nbuf + 1)  # xsq slot reader
                vector.wait_ge(s_dx[j], 16 * (i // nbuf + 1))
                xsq3 = xsq[j][:, :].rearrange("p (b c) -> p b c", b=t)
                xt3 = xt[j][:, :].rearrange("p (b d) -> p b d", b=t)
                vector.tensor_mul(xsq3[:, :, 0:D], xt3, xt3).then_inc(s_dve)

        @block.tensor
        def _(tensor):
            for i in range(iters):
                j = i % nbuf
                tensor.wait_ge(s_act, i + 1)
                tensor.wait_ge(s_dve, i + 1)
                tensor.wait_ge(s_dx[j], 16 * (i // nbuf + 1))
                last = None
                for b in range(t):
                    tensor.matmul(
                        acc_w[:, :],
                        lhsT=u2[j][:, b * K : (b + 1) * K],
                        rhs=xt[j][:, b * D : (b + 1) * D],
                        start=(i == 0 and b == 0),
                        stop=(i == iters - 1 and b == t - 1),
                    )
                    last = tensor.matmul(
                        acc_q[:, :],
                        lhsT=u2[j][:, b * K : (b + 1) * K],
                        rhs=xsq[j][:, b * w2 : (b + 1) * w2],
                        start=(i == 0 and b == 0),
                        stop=(i == iters - 1 and b == t - 1),
                    )
                last.then_inc(s_pe)

    return nc


def combine_host(parts, v):
    """Combine per-core [K, OUT_W] partials (scaled by USCALE^2) with v in
    float64 on the host."""
    acc = np.zeros((K, OUT_W), np.float64)
    for p in parts:
        acc += np.asarray(p, np.float64)
    acc /= USCALE * USCALE
    W = acc[:, :D]
    t1 = acc[:, D : 2 * D].sum()
    c = acc[:, 2 * D]
    v64 = np.asarray(v, np.float64)
    v2 = (v64 * v64).sum(axis=1)
    loss = t1 + (v2 * c).sum() - 2.0 * (W * v64).sum()
    return np.asarray(GAMMA * loss, dtype=np.float32)


def kernel(x, u, v):
    global LAST_RESULTS
    x = np.asarray(x)
    u = np.asarray(u)
    assert x.shape == (N, D) and u.shape == (N, K)
    x16 = np.ascontiguousarray(x.astype(np.float16))
    u16 = np.ascontiguousarray((np.asarray(u, np.float32) * USCALE).astype(np.float16))

    if "nc" not in _NC_CACHE:
        _NC_CACHE["nc"] = build_nc()
    nc = _NC_CACHE["nc"]

    in_maps = [
        {
            "x": x16[c * N_CORE : (c + 1) * N_CORE],
            "u": u16[c * N_CORE : (c + 1) * N_CORE],
        }
        for c in range(NCORES)
    ]
    LAST_RESULTS = run_bass_kernel_spmd(nc, in_maps, list(range(NCORES)))
    return combine_host([r["out"] for r in LAST_RESULTS.results], v)



# revision 2
# speedup vs baseline: 1.6596x; 1.6596x over previous
"""Trainium2 Bass kernel for the DeepFuzzyCMean loss.

loss = GAMMA * sum_{n,k} u[n,k]^2 * ||x[n] - v[k]||^2
     = GAMMA * ( t1 + sum_k c[k]*|v_k|^2 - 2*sum_{k,d} W[k,d]*v[k,d] )
  t1 = sum_n (sum_k u2[n,k]) * |x_n|^2,  c = colsum(u2),  W = u2^T @ x

Device formulation (per 128x(t*rows) tile, contraction over n in PSUM):
  xsq = [x*x | 1]  (ACT Square on blocks 0..NA-1, DVE mul on the rest;
                    ones cols preset once per buffer)
  MM1: acc_w[64,128]  += u2_pair^T @ x_pair    (W)      fp8 DoubleRow
  MM2: acc_q[64,129]  += u2_pair^T @ xsq_pair  ([t1|c]) fp8 DoubleRow
Host combines the per-core [64,257] partials with v in float64.

Wire format: x and u2=(u*USCALE)^2 cast to float8-e4m3 on host (memory-bound:
halves HBM traffic vs fp16; rel err ~5e-3). Shipping u pre-squared keeps both
ACT and DVE free to split the x*x work, which neither engine can cover alone
at fp8 rates. DoubleRow matmuls contract 256 rows/call (two adjacent row
blocks per partition); pairing is consistent on lhsT/rhs so the row-sum is
unchanged. t=32 consecutive rows map to one SBUF partition so every DMA is a
fully contiguous 4KB/partition block (full line rate, half the DMA count of
the fp16 kernel).

Raw-bass implementation (manual semaphores, standalone sequencer waits).
nbuf-deep buffering; data-parallel over N across 8 NeuronCores with a host
all-reduce.
"""

import sys
import types
from contextlib import ExitStack

import numpy as np
import ml_dtypes

import concourse.bass as bass
from concourse import mybir
from concourse.bass_utils import run_bass_kernel_spmd

# run_bass_kernel_spmd(trace=True) under axon imports antenv.axon_hooks,
# which this container lacks; stub it so a BASS_TRACE env var can't crash us.
try:
    import antenv.axon_hooks  # noqa: F401
except ImportError:
    try:
        import antenv

        _stub = types.ModuleType("antenv.axon_hooks")
        _stub.get_axon_ntff_profile_hook = lambda: None
        sys.modules["antenv.axon_hooks"] = _stub
        antenv.axon_hooks = _stub
    except ImportError:
        pass

GAMMA = 1e-06
N, K, D = 262144, 64, 128
NCORES = 8
N_CORE = N // NCORES
P = 128
OUT_W = 2 * D + 1  # [W | t1 block | c] = 257
USCALE = 64.0      # u pre-scale; partials carry USCALE^2 = 4096
F8NP = ml_dtypes.float8_e4m3

LAST_RESULTS = None
_NC_CACHE = {}


def build_nc(n_rows=N_CORE, t=32, nbuf=4, num_devices=NCORES, reps=1, na=17):
    """reps>1 repeats the full sweep inside one NEFF (re-reading the same
    DRAM) — used only for differential hardware timing. na = number of
    x*x blocks handled by ACT (rest go to DVE)."""
    assert n_rows % (P * t) == 0
    assert t % 2 == 0
    iters = (n_rows // (P * t)) * reps
    n_rows_data = n_rows
    assert iters >= nbuf
    f8 = mybir.dt.float8e4
    f32 = mybir.dt.float32
    w2 = D + 1
    nd0 = na  # first DVE block

    nc = bass.Bass("TRN2", num_devices=num_devices)
    x_d = nc.dram_tensor("x", [n_rows, D], f8, kind="ExternalInput")
    u_d = nc.dram_tensor("u", [n_rows, K], f8, kind="ExternalInput")
    out_d = nc.dram_tensor("out", [K, OUT_W], f32, kind="ExternalOutput")

    with ExitStack() as ctx:
        xt = [ctx.enter_context(nc.sbuf_tensor(f"xt{j}", [P, t * D], f8)) for j in range(nbuf)]
        u2 = [ctx.enter_context(nc.sbuf_tensor(f"u2{j}", [P, t * K], f8)) for j in range(nbuf)]
        xsq = [ctx.enter_context(nc.sbuf_tensor(f"xsq{j}", [P, t * w2], f8)) for j in range(nbuf)]
        res = ctx.enter_context(nc.sbuf_tensor("res", [K, OUT_W], f32))
        acc_w = ctx.enter_context(nc.psum_tensor([K, D], f32))
        acc_q = ctx.enter_context(nc.psum_tensor([K, w2], f32))

        s_dx = [ctx.enter_context(nc.semaphore(f"s_dx{j}")) for j in range(nbuf)]
        s_du = [ctx.enter_context(nc.semaphore(f"s_du{j}")) for j in range(nbuf)]
        s_act = ctx.enter_context(nc.semaphore("s_act"))
        s_dve = ctx.enter_context(nc.semaphore("s_dve"))
        s_pe = ctx.enter_context(nc.semaphore("s_pe"))
        s_res = ctx.enter_context(nc.semaphore("s_res"))
        s_do = ctx.enter_context(nc.semaphore("s_do"))

        block = ctx.enter_context(nc.Block())

        data_iters = n_rows_data // (P * t)

        @block.sync
        def _(sync):
            for i in range(iters):
                j = i % nbuf
                r = (i % data_iters) * P * t
                e = i // nbuf
                if i >= nbuf:
                    # slot j free: PE done with xt/u2/xsq, DVE+ACT done with
                    # xt of iteration i-nbuf; the slot's own previous DMAs
                    # completed long ago (consumers saw them) — these waits
                    # are immediately satisfied but keep the per-sem
                    # increments ordered for the race checker.
                    sync.wait_ge(s_pe, i - nbuf + 1)
                    sync.wait_ge(s_dve, i - nbuf + 1)
                    sync.wait_ge(s_act, i - nbuf + 1)
                    sync.wait_ge(s_dx[j], 16 * e)
                    sync.wait_ge(s_du[j], 16 * e)
                x_src = x_d[r : r + P * t, :].rearrange("(p b) d -> p (b d)", p=P)
                u_src = u_d[r : r + P * t, :].rearrange("(p b) k -> p (b k)", p=P)
                sync.dma_start(out=xt[j][:, :], in_=x_src).then_inc(s_dx[j], 16)
                sync.dma_start(out=u2[j][:, :], in_=u_src).then_inc(s_du[j], 16)
            sync.wait_ge(s_res, 1)
            sync.dma_start(out=out_d[:, :], in_=res[:, :]).then_inc(s_do, 16)
            sync.wait_ge(s_do, 16)

        @block.scalar
        def _(scalar):
            # ACT squares blocks [0, na); writes xsq in place (fp8 in/out).
            for i in range(iters):
                j = i % nbuf
                if i >= nbuf:
                    scalar.wait_ge(s_pe, i - nbuf + 1)  # xsq slot reader
                scalar.wait_ge(s_dx[j], 16 * (i // nbuf + 1))
                xsq3 = xsq[j][:, :].rearrange("p (b c) -> p b c", b=t)
                xt3 = xt[j][:, :].rearrange("p (b d) -> p b d", b=t)
                scalar.activation(
                    out=xsq3[:, 0:na, 0:D],
                    in_=xt3[:, 0:na, :],
                    func=mybir.ActivationFunctionType.Square,
                ).then_inc(s_act)
            # tail: psum -> sbuf -> (sync engine DMAs it out)
            scalar.wait_ge(s_pe, iters)
            scalar.copy(res[:, 0:D], acc_w[:, :])
            scalar.copy(res[:, D:OUT_W], acc_q[:, :]).then_inc(s_res)

        @block.vector
        def _(vector):
            # ones columns are static: set once per buffer, never overwritten
            for j in range(nbuf):
                xsq3 = xsq[j][:, :].rearrange("p (b c) -> p b c", b=t)
                vector.memset(xsq3[:, :, D : D + 1], 1.0)
            # DVE squares blocks [na, t)
            for i in range(iters):
                j = i % nbuf
                if i >= nbuf:
                    vector.wait_ge(s_pe, i - nbuf + 1)  # xsq slot reader
                vector.wait_ge(s_dx[j], 16 * (i // nbuf + 1))
                xsq3 = xsq[j][:, :].rearrange("p (b c) -> p b c", b=t)
                xt3 = xt[j][:, :].rearrange("p (b d) -> p b d", b=t)
                vector.tensor_mul(
                    xsq3[:, nd0:t, 0:D], xt3[:, nd0:t, :], xt3[:, nd0:t, :]
                ).then_inc(s_dve)

        @block.tensor
        def _(tensor):
            for i in range(iters):
                j = i % nbuf
                tensor.wait_ge(s_act, i + 1)
                tensor.wait_ge(s_dve, i + 1)
                tensor.wait_ge(s_dx[j], 16 * (i // nbuf + 1))
                tensor.wait_ge(s_du[j], 16 * (i // nbuf + 1))
                last = None
                for b in range(t // 2):
                    lhsT = u2[j][:, 2 * b * K : (2 * b + 2) * K].rearrange(
                        "p (two k) -> p two k", two=2
                    )
                    tensor.matmul(
                        acc_w[:, :],
                        lhsT=lhsT,
                        rhs=xt[j][:, 2 * b * D : (2 * b + 2) * D].rearrange(
                            "p (two d) -> p two d", two=2
                        ),
                        start=(i == 0 and b == 0),
                        stop=(i == iters - 1 and b == t // 2 - 1),
                        perf_mode=mybir.MatmulPerfMode.DoubleRow,
                    )
                    last = tensor.matmul(
                        acc_q[:, :],
                        lhsT=lhsT,
                        rhs=xsq[j][:, 2 * b * w2 : (2 * b + 2) * w2].rearrange(
                            "p (two c) -> p two c", two=2
                        ),
                        start=(i == 0 and b == 0),
                        stop=(i == iters - 1 and b == t // 2 - 1),
                        perf_mode=mybir.MatmulPerfMode.DoubleRow,
                    )
                last.then_inc(s_pe)

    return nc


def combine_host(parts, v):
    """Combine per-core [K, OUT_W] partials (scaled by USCALE^2) with v in
    float64 on the host."""
    acc = np.zeros((K, OUT_W), np.float64)
    for p in parts:
        acc += np.asarray(p, np.float64)
    acc /= USCALE * USCALE
    W = acc[:, :D]
    t1 = acc[:, D : 2 * D].sum()
    c = acc[:, 2 * D]
    v64 = np.asarray(v, np.float64)
    v2 = (v64 * v64).sum(axis=1)
    loss = t1 + (v2 * c).sum() - 2.0 * (W * v64).sum()
    return np.asarray(GAMMA * loss, dtype=np.float32)


def kernel(x, u, v):
    global LAST_RESULTS
    x = np.asarray(x)
    u = np.asarray(u)
    assert x.shape == (N, D) and u.shape == (N, K)
    x8 = np.ascontiguousarray(x.astype(F8NP))
    u32 = np.asarray(u, np.float32) * USCALE
    u28 = np.ascontiguousarray((u32 * u32).astype(F8NP))

    if "nc" not in _NC_CACHE:
        _NC_CACHE["nc"] = build_nc()
    nc = _NC_CACHE["nc"]

    in_maps = [
        {
            "x": x8[c * N_CORE : (c + 1) * N_CORE],
            "u": u28[c * N_CORE : (c + 1) * N_CORE],
        }
        for c in range(NCORES)
    ]
    LAST_RESULTS = run_bass_kernel_spmd(nc, in_maps, list(range(NCORES)))
    return combine_host([r["out"] for r in LAST_RESULTS.results], v)


# revision 11
# speedup vs baseline: 1.7668x; 1.0646x over previous
"""Trainium2 Bass kernel for the DeepFuzzyCMean loss.

loss = GAMMA * sum_{n,k} u[n,k]^2 * ||x[n] - v[k]||^2
     = GAMMA * ( t1 + sum_k c[k]*|v_k|^2 - 2*sum_{k,d} W[k,d]*v[k,d] )
  t1 = sum_n (sum_k u2[n,k]) * |x_n|^2,  c = colsum(u2),  W = u2^T @ x

Device formulation (per 128x(t*rows) tile, contraction over n in PSUM):
  xsq = [x*x | 1]  (x*x split across ACT Square / DVE mul / Pool mul so each
                    tile's squares finish inside the DMA arrival cadence;
                    ones cols preset once per buffer)
  MM1: acc_w[64,128]  += u2_pair^T @ x_pair    (W)      fp8 DoubleRow
  MM2: acc_q[64,129]  += u2_pair^T @ xsq_pair  ([t1|c]) fp8 DoubleRow
Host combines the per-core [64,257] partials with v in float64.

Wire format: x and u2=(u*USCALE)^2 cast to float8-e4m3 on host (memory-bound:
halves HBM traffic vs fp16; rel err ~5e-3). Shipping u pre-squared keeps ACT,
DVE and Pool free to split the x*x work. DoubleRow matmuls contract 256
rows/call (two adjacent row blocks per partition); pairing is consistent on
lhsT/rhs so the row-sum is unchanged. t=32 consecutive rows map to one SBUF
partition so every DMA is a fully contiguous 4KB/partition block.

The last iteration is phase-split (x sub-DMAs ordered before the u2 ones) so
the final squares+matmuls chain off earlier data arrivals; PSUM->SBUF result
copies run on ACT and DVE in parallel.

Raw-bass implementation (manual semaphores, standalone sequencer waits).
nbuf-deep buffering; data-parallel over N across 8 NeuronCores with a host
all-reduce.
"""

import sys
import types
from contextlib import ExitStack

import numpy as np
import ml_dtypes

import concourse.bass as bass
from concourse import mybir
from concourse.bass_utils import run_bass_kernel_spmd

# run_bass_kernel_spmd(trace=True) under axon imports antenv.axon_hooks,
# which this container lacks; stub it so a BASS_TRACE env var can't crash us.
try:
    import antenv.axon_hooks  # noqa: F401
except ImportError:
    try:
        import antenv

        _stub = types.ModuleType("antenv.axon_hooks")
        _stub.get_axon_ntff_profile_hook = lambda: None
        sys.modules["antenv.axon_hooks"] = _stub
        antenv.axon_hooks = _stub
    except ImportError:
        pass

GAMMA = 1e-06
N, K, D = 262144, 64, 128
NCORES = 8
N_CORE = N // NCORES
P = 128
OUT_W = 2 * D + 1  # [W | t1 block | c] = 257
USCALE = 64.0      # u pre-scale; partials carry USCALE^2 = 4096
F8NP = ml_dtypes.float8_e4m3

LAST_RESULTS = None
_NC_CACHE = {}


def build_nc(
    n_rows=N_CORE,
    t=32,
    nbuf=4,
    num_devices=NCORES,
    reps=1,
    shares=(14, 12),
    tail=((16, 7, 6), (16, 7, 6)),
):
    """reps>1 repeats the full sweep inside one NEFF (re-reading the same
    DRAM) — used only for differential hardware timing.
    shares = (act, dve) x*x blocks per steady iteration; Pool takes the rest.
    tail = ((blocks, act, dve), ...) phases for the LAST iteration: its DMAs
    are split, all x sub-DMAs ahead of the u2 ones, so the final squares +
    matmuls chain off earlier data arrivals."""
    assert n_rows % (P * t) == 0
    assert t % 2 == 0
    iters = (n_rows // (P * t)) * reps
    n_rows_data = n_rows
    assert iters >= nbuf
    assert sum(b for b, _, _ in tail) == t and all(b % 2 == 0 for b, _, _ in tail)
    nph = len(tail)
    sa, sv = shares
    f8 = mybir.dt.float8e4
    f32 = mybir.dt.float32
    w2 = D + 1

    nc = bass.Bass("TRN2", num_devices=num_devices)
    x_d = nc.dram_tensor("x", [n_rows, D], f8, kind="ExternalInput")
    u_d = nc.dram_tensor("u", [n_rows, K], f8, kind="ExternalInput")
    out_d = nc.dram_tensor("out", [K, OUT_W], f32, kind="ExternalOutput")

    with ExitStack() as ctx:
        xt = [ctx.enter_context(nc.sbuf_tensor(f"xt{j}", [P, t * D], f8)) for j in range(nbuf)]
        u2 = [ctx.enter_context(nc.sbuf_tensor(f"u2{j}", [P, t * K], f8)) for j in range(nbuf)]
        xsq = [ctx.enter_context(nc.sbuf_tensor(f"xsq{j}", [P, t * w2], f8)) for j in range(nbuf)]
        res = ctx.enter_context(nc.sbuf_tensor("res", [K, OUT_W], f32))
        acc_w = ctx.enter_context(nc.psum_tensor([K, D], f32))
        acc_q = ctx.enter_context(nc.psum_tensor([K, w2], f32))

        s_dx = [ctx.enter_context(nc.semaphore(f"s_dx{j}")) for j in range(nbuf)]
        s_du = [ctx.enter_context(nc.semaphore(f"s_du{j}")) for j in range(nbuf)]
        s_tx = [ctx.enter_context(nc.semaphore(f"s_tx{p}")) for p in range(nph)]
        s_tu = [ctx.enter_context(nc.semaphore(f"s_tu{p}")) for p in range(nph)]
        s_act = ctx.enter_context(nc.semaphore("s_act"))
        s_dve = ctx.enter_context(nc.semaphore("s_dve"))
        s_pool = ctx.enter_context(nc.semaphore("s_pool"))
        s_pe = ctx.enter_context(nc.semaphore("s_pe"))
        s_res = ctx.enter_context(nc.semaphore("s_res"))
        s_do = ctx.enter_context(nc.semaphore("s_do"))

        block = ctx.enter_context(nc.Block())

        data_iters = n_rows_data // (P * t)

        def views(j):
            xsq3 = xsq[j][:, :].rearrange("p (b c) -> p b c", b=t)
            xt3 = xt[j][:, :].rearrange("p (b d) -> p b d", b=t)
            return xsq3, xt3

        @block.sync
        def _(sync):
            for i in range(iters):
                j = i % nbuf
                r = (i % data_iters) * P * t
                e = i // nbuf
                if i >= nbuf:
                    # slot j free: PE done with xt/u2/xsq, the square engines
                    # done with xt of iteration i-nbuf; the slot's own
                    # previous DMAs completed long ago (consumers saw them) —
                    # these waits are immediately satisfied but keep the
                    # per-sem increments ordered for the race checker.
                    sync.wait_ge(s_pe, i - nbuf + 1)
                    sync.wait_ge(s_dve, i - nbuf + 1)
                    sync.wait_ge(s_act, i - nbuf + 1)
                    sync.wait_ge(s_pool, i - nbuf + 1)
                    sync.wait_ge(s_dx[j], 16 * e)
                    sync.wait_ge(s_du[j], 16 * e)
                if i < iters - 1:
                    x_src = x_d[r : r + P * t, :].rearrange("(p b) d -> p (b d)", p=P)
                    u_src = u_d[r : r + P * t, :].rearrange("(p b) k -> p (b k)", p=P)
                    sync.dma_start(out=xt[j][:, :], in_=x_src).then_inc(s_dx[j], 16)
                    sync.dma_start(out=u2[j][:, :], in_=u_src).then_inc(s_du[j], 16)
                else:
                    # tail iteration: phase-split DMAs, all x sub-DMAs ahead
                    # of the u2 ones (PE, the u2 consumer, runs last anyway).
                    x_src = x_d[r : r + P * t, :].rearrange("(p b) d -> p b d", p=P)
                    u_src = u_d[r : r + P * t, :].rearrange("(p b) k -> p b k", p=P)
                    xt3, u23 = (
                        xt[j][:, :].rearrange("p (b d) -> p b d", b=t),
                        u2[j][:, :].rearrange("p (b k) -> p b k", b=t),
                    )
                    off = 0
                    for p, (bp, _, _) in enumerate(tail):
                        sync.dma_start(
                            out=xt3[:, off : off + bp, :],
                            in_=x_src[:, off : off + bp, :],
                        ).then_inc(s_tx[p], 16)
                        off += bp
                    off = 0
                    for p, (bp, _, _) in enumerate(tail):
                        sync.dma_start(
                            out=u23[:, off : off + bp, :],
                            in_=u_src[:, off : off + bp, :],
                        ).then_inc(s_tu[p], 16)
                        off += bp
            sync.wait_ge(s_res, 2)
            sync.dma_start(out=out_d[:, :], in_=res[:, :]).then_inc(s_do, 16)
            sync.wait_ge(s_do, 16)

        @block.scalar
        def _(scalar):
            # ACT squares the first sa blocks of each tile.
            for i in range(iters):
                j = i % nbuf
                if i >= nbuf:
                    scalar.wait_ge(s_pe, i - nbuf + 1)  # xsq slot reader
                xsq3, xt3 = views(j)
                if i < iters - 1:
                    scalar.wait_ge(s_dx[j], 16 * (i // nbuf + 1))
                    scalar.activation(
                        out=xsq3[:, 0:sa, 0:D],
                        in_=xt3[:, 0:sa, :],
                        func=mybir.ActivationFunctionType.Square,
                    ).then_inc(s_act)
                else:
                    off = 0
                    for p, (bp, ap_, _) in enumerate(tail):
                        scalar.wait_ge(s_tx[p], 16)
                        scalar.activation(
                            out=xsq3[:, off : off + ap_, 0:D],
                            in_=xt3[:, off : off + ap_, :],
                            func=mybir.ActivationFunctionType.Square,
                        ).then_inc(s_act)
                        off += bp
            # tail: psum -> sbuf (acc_q goes via DVE in parallel) ->
            # (sync engine DMAs res out)
            scalar.wait_ge(s_pe, iters - 1 + nph)
            scalar.copy(res[:, 0:D], acc_w[:, :]).then_inc(s_res)

        @block.vector
        def _(vector):
            # ones columns are static: set once per buffer, never overwritten
            for j in range(nbuf):
                xsq3, _ = views(j)
                vector.memset(xsq3[:, :, D : D + 1], 1.0)
            # DVE squares blocks [sa, sa+sv) of steady tiles
            for i in range(iters):
                j = i % nbuf
                if i >= nbuf:
                    vector.wait_ge(s_pe, i - nbuf + 1)  # xsq slot reader
                xsq3, xt3 = views(j)
                if i < iters - 1:
                    vector.wait_ge(s_dx[j], 16 * (i // nbuf + 1))
                    vector.tensor_mul(
                        xsq3[:, sa : sa + sv, 0:D],
                        xt3[:, sa : sa + sv, :],
                        xt3[:, sa : sa + sv, :],
                    ).then_inc(s_dve)
                else:
                    off = 0
                    for p, (bp, ap_, vp_) in enumerate(tail):
                        vector.wait_ge(s_tx[p], 16)
                        lo, hi = off + ap_, off + ap_ + vp_
                        vector.tensor_mul(
                            xsq3[:, lo:hi, 0:D], xt3[:, lo:hi, :], xt3[:, lo:hi, :]
                        ).then_inc(s_dve)
                        off += bp
            # parallel tail copy: acc_q -> res while ACT copies acc_w
            vector.wait_ge(s_pe, iters - 1 + nph)
            vector.tensor_copy(res[:, D:OUT_W], acc_q[:, :]).then_inc(s_res)

        @block.gpsimd
        def _(gp):
            # Pool squares the trailing blocks of each tile.
            for i in range(iters):
                j = i % nbuf
                if i >= nbuf:
                    gp.wait_ge(s_pe, i - nbuf + 1)  # xsq slot reader
                xsq3, xt3 = views(j)
                if i < iters - 1:
                    gp.wait_ge(s_dx[j], 16 * (i // nbuf + 1))
                    gp.tensor_mul(
                        xsq3[:, sa + sv : t, 0:D],
                        xt3[:, sa + sv : t, :],
                        xt3[:, sa + sv : t, :],
                    ).then_inc(s_pool)
                else:
                    off = 0
                    for p, (bp, ap_, vp_) in enumerate(tail):
                        gp.wait_ge(s_tx[p], 16)
                        lo, hi = off + ap_ + vp_, off + bp
                        gp.tensor_mul(
                            xsq3[:, lo:hi, 0:D], xt3[:, lo:hi, :], xt3[:, lo:hi, :]
                        ).then_inc(s_pool)
                        off += bp

        @block.tensor
        def _(tensor):
            def mm_pairs(j, b0, b1, start, stop):
                last = None
                for b in range(b0 // 2, b1 // 2):
                    lhsT = u2[j][:, 2 * b * K : (2 * b + 2) * K].rearrange(
                        "p (two k) -> p two k", two=2
                    )
                    tensor.matmul(
                        acc_w[:, :],
                        lhsT=lhsT,
                        rhs=xt[j][:, 2 * b * D : (2 * b + 2) * D].rearrange(
                            "p (two d) -> p two d", two=2
                        ),
                        start=(start and b == b0 // 2),
                        stop=(stop and b == b1 // 2 - 1),
                        perf_mode=mybir.MatmulPerfMode.DoubleRow,
                    )
                    last = tensor.matmul(
                        acc_q[:, :],
                        lhsT=lhsT,
                        rhs=xsq[j][:, 2 * b * w2 : (2 * b + 2) * w2].rearrange(
                            "p (two c) -> p two c", two=2
                        ),
                        start=(start and b == b0 // 2),
                        stop=(stop and b == b1 // 2 - 1),
                        perf_mode=mybir.MatmulPerfMode.DoubleRow,
                    )
                return last

            nacts = 0  # completed per-square-engine increments so far
            for i in range(iters):
                j = i % nbuf
                if i < iters - 1:
                    # x-arrival is implied by the square sems (the three
                    # square engines jointly read every x block first)
                    tensor.wait_ge(s_du[j], 16 * (i // nbuf + 1))
                    tensor.wait_ge(s_pool, nacts + 1)
                    tensor.wait_ge(s_dve, nacts + 1)
                    tensor.wait_ge(s_act, nacts + 1)
                    mm_pairs(j, 0, t, i == 0, False).then_inc(s_pe)
                    nacts += 1
                else:
                    off = 0
                    for p, (bp, _, _) in enumerate(tail):
                        tensor.wait_ge(s_tu[p], 16)
                        tensor.wait_ge(s_pool, nacts + 1)
                        tensor.wait_ge(s_dve, nacts + 1)
                        tensor.wait_ge(s_act, nacts + 1)
                        mm_pairs(
                            j, off, off + bp, False, p == nph - 1
                        ).then_inc(s_pe)
                        nacts += 1
                        off += bp

    return nc


def combine_host(parts, v):
    """Combine per-core [K, OUT_W] partials (scaled by USCALE^2) with v in
    float64 on the host."""
    acc = np.zeros((K, OUT_W), np.float64)
    for p in parts:
        acc += np.asarray(p, np.float64)
    acc /= USCALE * USCALE
    W = acc[:, :D]
    t1 = acc[:, D : 2 * D].sum()
    c = acc[:, 2 * D]
    v64 = np.asarray(v, np.float64)
    v2 = (v64 * v64).sum(axis=1)
    loss = t1 + (v2 * c).sum() - 2.0 * (W * v64).sum()
    return np.asarray(GAMMA * loss, dtype=np.float32)


def kernel(x, u, v):
    global LAST_RESULTS
    x = np.asarray(x)
    u = np.asarray(u)
    assert x.shape == (N, D) and u.shape == (N, K)
    x8 = np.ascontiguousarray(x.astype(F8NP))
    u32 = np.asarray(u, np.float32) * USCALE
    u28 = np.ascontiguousarray((u32 * u32).astype(F8NP))

    if "nc" not in _NC_CACHE:
        _NC_CACHE["nc"] = build_nc()
    nc = _NC_CACHE["nc"]

    in_maps = [
        {
            "x": x8[c * N_CORE : (c + 1) * N_CORE],
            "u": u28[c * N_CORE : (c + 1) * N_CORE],
        }
        for c in range(NCORES)
    ]
    LAST_RESULTS = run_bass_kernel_spmd(nc, in_maps, list(range(NCORES)))
    return combine_host([r["out"] for r in LAST_RESULTS.results], v)
